# revision 1
# baseline (speedup 1.0000x reference)
"""Multi-head attention (B=4, S=2048, D=1024, H=16) on 8 Trainium2 cores.

Sharding: DP=4 over batch x TP=2 over heads (8 heads/core). Each core:
  - projects its batch's Q/K/V into per-head-pair transposed layouts
    (qT/kT: [dh, S] with dh on partitions; v: [S, dh])
  - flash-style attention without max-subtraction (scores ~ N(0,1)):
    S^T = kT.T-chunks @ qT (row-tiled pairs of heads, K=64 each),
    P^T = exp(S^T/8) in bf16,
    O^T = [v | 1].T @ P^T  (ones column fuses the softmax denominator
    into row 64 of the PV output),
    normalize via denominator broadcast (DMA through DRAM) + reciprocal.
  - output projection partial Y_g = A_g @ Wo_g.T  (f32r)
Host sums the two TP partials per batch and adds bo + Wo @ bv (the v-bias
commutes through the normalized softmax).
"""

import os
import sys

sys.path.insert(0, "/opt/trn_rl_repo")
os.environ.setdefault("MYCRO_LOCAL_CACHE", "1")

import numpy as np
import ml_dtypes
import concourse.bass as bass  # noqa: F401  (Bass types via bacc)
import concourse.mybir as mybir
import concourse.tile as tile
from concourse import bacc
from concourse.bass_utils import run_bass_kernel_spmd
from contextlib import ExitStack

f32 = mybir.dt.float32
f32r = mybir.dt.float32r
bf16 = mybir.dt.bfloat16
AF = mybir.ActivationFunctionType
MUL = mybir.AluOpType.mult

B, S, D = 4, 2048, 1024
H = 16
DH = 64
NCORES = 8
G_HEADS = 512  # head dims per core (8 heads)


def build():
    nc = bacc.Bacc(None, target_bir_lowering=False)

    QT = nc.dram_tensor("QT", [D, S], f32r, kind="ExternalInput")
    KT = nc.dram_tensor("KT", [D, S], bf16, kind="ExternalInput")
    VT = nc.dram_tensor("VT", [D, S], bf16, kind="ExternalInput")
    WqT = nc.dram_tensor("WqT", [D, G_HEADS], f32r, kind="ExternalInput")
    WkT = nc.dram_tensor("WkT", [D, G_HEADS], bf16, kind="ExternalInput")
    WvT = nc.dram_tensor("WvT", [D, G_HEADS], bf16, kind="ExternalInput")
    WoT = nc.dram_tensor("WoT", [G_HEADS, D], f32r, kind="ExternalInput")
    bqp = nc.dram_tensor("bqp", [128, 4], f32, kind="ExternalInput")
    bkp = nc.dram_tensor("bkp", [128, 4], f32, kind="ExternalInput")
    Y = nc.dram_tensor("Y", [S, D], f32, kind="ExternalOutput")

    with tile.TileContext(nc) as tc, ExitStack() as top:
        qkpool = top.enter_context(tc.tile_pool(name="qk", bufs=1))
        vpool = top.enter_context(tc.tile_pool(name="vp", bufs=1))
        atpool = top.enter_context(tc.tile_pool(name="at", bufs=1))
        wq = top.enter_context(tc.tile_pool(name="wq", bufs=1))
        xq = top.enter_context(tc.tile_pool(name="xq", bufs=3))
        xv = top.enter_context(tc.tile_pool(name="xv", bufs=2))
        cst = top.enter_context(tc.tile_pool(name="cst", bufs=1))
        ppool = top.enter_context(tc.tile_pool(name="pP", bufs=4))
        oev = top.enter_context(tc.tile_pool(name="oev", bufs=2))
        dbp = top.enter_context(tc.tile_pool(name="dbp", bufs=2))
        osc = top.enter_context(tc.tile_pool(name="osc", bufs=2))
        yev = top.enter_context(tc.tile_pool(name="yev", bufs=3))
        drp = top.enter_context(tc.tile_pool(name="drp", bufs=4, space="DRAM"))
        pps = top.enter_context(tc.tile_pool(name="pps", bufs=2, space="PSUM"))
        spool = top.enter_context(tc.tile_pool(name="sS", bufs=2, space="PSUM"))
        opool = top.enter_context(tc.tile_pool(name="sO", bufs=2, space="PSUM"))

        # per-quarter q tiles, whole-kT tiles, per-seqtile v, per-(hp,qp) A^T
        qq_t = [
            [qkpool.tile([128, 512], bf16, tag=f"qq{i}_{j}", name=f"qq{i}_{j}") for j in range(4)]
            for i in range(4)
        ]
        kT_t = [qkpool.tile([128, S], bf16, tag=f"kT{i}", name=f"kT{i}") for i in range(4)]
        v_st = [vpool.tile([128, 8 * 65], bf16, tag=f"v{i}", name=f"v{i}") for i in range(16)]
        AT_q = [
            [atpool.tile([128, 512], f32r, tag=f"AT{i}_{j}", name=f"AT{i}_{j}") for j in range(4)]
            for i in range(4)
        ]

        bq_sb = cst.tile([128, 4], f32, tag="bq")
        bk_sb = cst.tile([128, 4], f32, tag="bk")
        nc.sync.dma_start(bq_sb[:], bqp[:, :])
        nc.sync.dma_start(bk_sb[:], bkp[:, :])
        Wk_dc = [wq.tile([128, G_HEADS], bf16, tag=f"Wk{dc}", name=f"Wk{dc}") for dc in range(8)]
        Wq_dc = [wq.tile([128, G_HEADS], f32r, tag=f"Wq{dc}", name=f"Wq{dc}") for dc in range(8)]
        WvT_sb = wq.tile([128, 8, G_HEADS], bf16, tag="Wv")
        WoT_sb = wq.tile([128, 4, D], f32r, tag="Wo")
        ksrc = WkT.ap().rearrange("(d p) c -> p d c", p=128)
        qsrc = WqT.ap().rearrange("(d p) c -> p d c", p=128)
        for dc in range(8):
            nc.sync.dma_start(Wk_dc[dc][:], ksrc[:, dc, :])
        for dc in range(8):
            nc.gpsimd.dma_start(Wq_dc[dc][:], qsrc[:, dc, :])
        nc.gpsimd.dma_start(WvT_sb[:], WvT.ap().rearrange("(d p) c -> p d c", p=128))
        nc.gpsimd.dma_start(WoT_sb[:], WoT.ap().rearrange("(d p) n -> p d n", p=128))

        # warm the exp table set early (one-time ~2.7us load)
        warm = cst.tile([128, 8], f32, tag="warm")
        nc.vector.memset(warm[:], 0.0)
        nc.scalar.activation(warm[:], warm[:], AF.Exp)

        def qk_proj(XTd, W_dc, b_sb, scs, dest_fn, xdt=f32r):
            xsrc = XTd.ap().rearrange("(d p) s -> p d s", p=128)
            for sc in scs:
                halves = []
                for dh2 in range(2):
                    xt = xq.tile([128, 4, 512], xdt, tag="xt")
                    nc.sync.dma_start(
                        xt[:],
                        xsrc[:, dh2 * 4 : (dh2 + 1) * 4, sc * 512 : (sc + 1) * 512],
                    )
                    halves.append(xt)
                for hp in range(4):
                    ps = pps.tile([128, 512], f32, tag="ps")
                    for dc in range(8):
                        nc.tensor.matmul(
                            ps[:],
                            W_dc[dc][:, hp * 128 : (hp + 1) * 128],
                            halves[dc // 4][:, dc % 4, :],
                            start=(dc == 0),
                            stop=(dc == 7),
                        )
                    nc.vector.tensor_scalar_add(
                        dest_fn(hp, sc), ps[:], b_sb[:, hp : hp + 1]
                    )

        # k projection (attention needs full kT), then q quarter 0, then v,
        # then remaining q quarters
        qk_proj(KT, Wk_dc, bk_sb, range(4), lambda hp, sc: kT_t[hp][:, sc * 512 : (sc + 1) * 512], xdt=bf16)
        qk_proj(QT, Wq_dc, bq_sb, [0], lambda hp, sc: qq_t[hp][sc][:])

        vsrc = VT.ap().rearrange("(d p) s -> p d s", p=128)
        for st in range(16):
            xvt = xv.tile([128, 8, 128], bf16, tag="xv")
            nc.sync.dma_start(xvt[:], vsrc[:, :, st * 128 : (st + 1) * 128])
            ps = pps.tile([128, 512], f32, tag="ps")
            for dc in range(8):
                nc.tensor.matmul(
                    ps[:], xvt[:, dc, :], WvT_sb[:, dc, :], start=(dc == 0), stop=(dc == 7)
                )
            vd = v_st[st][:].rearrange("p (h c) -> p h c", c=65)
            nc.vector.tensor_copy(
                vd[:, :, 0:64], ps[:].rearrange("p (h c) -> p h c", c=64)
            )
            nc.vector.memset(vd[:, :, 64:65], 1.0)

        qk_proj(QT, Wq_dc, bq_sb, [1, 2, 3], lambda hp, sc: qq_t[hp][sc][:])

        # attention, quarter-outer so the output projection streams per quarter
        def emit_outproj(qp):
            for q4 in range(4):
                qt_g = qp * 4 + q4
                for nh in range(2):
                    ps = pps.tile([128, 512], f32, tag="ps")
                    for hp in range(4):
                        nc.tensor.matmul(
                            ps[:],
                            AT_q[hp][qp][:, q4 * 128 : (q4 + 1) * 128],
                            WoT_sb[:, hp, nh * 512 : (nh + 1) * 512],
                            start=(hp == 0),
                            stop=(hp == 3),
                        )
                    ye = yev.tile([128, 512], f32, tag="ye")
                    nc.vector.tensor_copy(ye[:], ps[:])
                    nc.sync.dma_start(
                        Y[qt_g * 128 : (qt_g + 1) * 128, nh * 512 : (nh + 1) * 512],
                        ye[:],
                    )

        for qp in range(4):
            for hp in range(4):
                kt = kT_t[hp]
                qtile = qq_t[hp][qp]
                O_t = [
                    opool.tile([128, 512], f32, tag="O", name="O0"),
                    opool.tile([128, 512], f32, tag="O", name="O1"),
                ]
                for kc in range(16):
                    S_big = spool.tile([128, 1024], f32, tag="S", name="S")
                    for hloc in range(2):
                        nc.tensor.matmul(
                            S_big[:, hloc * 512 : (hloc + 1) * 512],
                            kt[hloc * 64 : hloc * 64 + 64, kc * 128 : (kc + 1) * 128],
                            qtile[hloc * 64 : hloc * 64 + 64, :],
                            start=True,
                            stop=True,
                        )
                    P_big = ppool.tile([128, 1024], bf16, tag="P", name="P")
                    nc.scalar.activation(P_big[:], S_big[:], AF.Exp, scale=0.125)
                    for hloc in range(2):
                        lv = v_st[kc][:, (2 * hp + hloc) * 65 : (2 * hp + hloc) * 65 + 65]
                        nc.tensor.matmul(
                            O_t[hloc][0:65, :],
                            lv,
                            P_big[:, hloc * 512 : (hloc + 1) * 512],
                            start=(kc == 0),
                            stop=(kc == 15),
                        )
                for hloc in range(2):
                    ov = oev.tile([128, 512], f32, tag="oev")
                    nc.vector.tensor_copy(ov[0:65, :], O_t[hloc][0:65, :])
                    dr_t = drp.tile([1, 512], f32)
                    nc.sync.dma_start(dr_t[:, :], ov[64:65, :])
                    db_t = dbp.tile([128, 512], f32, tag="db")
                    nc.sync.dma_start(db_t[:], dr_t[0:1, :].to_broadcast([128, 512]))
                    rc_t = dbp.tile([128, 512], f32, tag="rc")
                    nc.vector.reciprocal_approx_fast(rc_t[:], db_t[:])
                    dst = AT_q[hp][qp][hloc * 64 : hloc * 64 + 64, :]
                    if hloc == 0:
                        nc.vector.tensor_tensor(dst, ov[0:64, :], rc_t[0:64, :], MUL)
                    else:
                        sc_t = osc.tile([128, 512], f32r, tag="osc")
                        nc.vector.tensor_tensor(
                            sc_t[0:64, :], ov[0:64, :], rc_t[0:64, :], MUL
                        )
                        nc.sync.dma_start(dst, sc_t[0:64, :])

            # emit the PREVIOUS quarter's output projection so it does not
            # preempt this quarter's score matmuls at the boundary
            if qp > 0:
                emit_outproj(qp - 1)
        emit_outproj(3)

    nc.compile()
    return nc


_NC = None


def _get_nc():
    global _NC
    if _NC is None:
        _NC = build()
    return _NC


def _prep_core(Q, K, V, Wq, bq, Wk, bk, Wv, Wo, b, g):
    c = np.ascontiguousarray
    hs = slice(g * G_HEADS, (g + 1) * G_HEADS)
    return {
        "QT": c(Q[b].T),
        "KT": c(K[b].T.astype(ml_dtypes.bfloat16)),
        "VT": c(V[b].T.astype(ml_dtypes.bfloat16)),
        "WqT": c(Wq[hs, :].T),
        "WkT": c(Wk[hs, :].T.astype(ml_dtypes.bfloat16)),
        "WvT": c(Wv[hs, :].T.astype(ml_dtypes.bfloat16)),
        "WoT": c(Wo[:, hs].T),
        "bqp": c(bq[hs].reshape(4, 128).T),
        "bkp": c(bk[hs].reshape(4, 128).T),
    }


def kernel(Q, K, V, Wq, bq, Wk, bk, Wv, bv, Wo, bo, _want_trace=False):
    Q, K, V = (np.asarray(x, np.float32) for x in (Q, K, V))
    Wq, bq, Wk, bk, Wv, bv, Wo, bo = (
        np.asarray(x, np.float32) for x in (Wq, bq, Wk, bk, Wv, bv, Wo, bo)
    )
    nc = _get_nc()
    in_maps = [
        _prep_core(Q, K, V, Wq, bq, Wk, bk, Wv, Wo, b=c % 4, g=c // 4)
        for c in range(NCORES)
    ]
    res = run_bass_kernel_spmd(
        nc, in_maps, core_ids=list(range(NCORES)), trace=_want_trace
    )
    out = np.zeros((B, S, D), np.float32)
    for c in range(NCORES):
        out[c % 4] += res.results[c]["Y"]
    out += (bo + Wo.astype(np.float64) @ bv.astype(np.float64)).astype(np.float32)[
        None, None, :
    ]
    if _want_trace:
        kernel.last_exec_time_ns = res.exec_time_ns
        kernel.last_trace = res.instructions_and_trace
    return out



# revision 13
# speedup vs baseline: 1.2482x; 1.2482x over previous
"""Multi-head attention (B=4, S=2048, D=1024, H=16) on 8 Trainium2 cores.

Sharding: DP=4 over batch x TP=2 over heads (8 heads/core). Each core:
  - loads X^T inputs once as big-element DMAs (bf16), projects Q/K/V into
    per-head-pair transposed layouts (qT/kT: [dh, S] with dh on partitions;
    v: [S, dh])
  - flash-style attention without max-subtraction (scores ~ N(0,1)):
    S^T = kT.T-chunks @ qT (row-group-concurrent pairs of heads, K=64 each),
    P^T = exp(S^T/8) in bf16 on the scalar engine's exp LUT (fp8 P/v were
    tried and are categorically too noisy: every tensor here is a
    normalized Gaussian, so matmul outputs carry the full per-element
    relative quantization noise of their inputs — fp8's ~3.6% exceeds the
    accuracy budget while bf16's ~0.4% fits).
    O^T = [v | 1].T @ P^T (the ones column fuses the softmax denominator
    into row 64 of the PV output), normalize by evicting O to SBUF,
    round-tripping the denominator row through DRAM as a broadcast,
    reciprocal on 64 partitions (the 1-partition fast-reciprocal is
    broken on HW), then multiply
  - output projection partial Y_g = A_g @ Wo_g.T (bf16), Y emitted bf16
  - Q quarters 1-3 and the previous quarter's output projection are
    emitted interleaved into the exp-bound attention loop so the Tile
    scheduler cannot starve the scalar engine with projection bursts.
Host sums the two TP partials per batch and adds bo + Wo @ bv (the v-bias
commutes through the normalized softmax).
"""

import os
import sys

sys.path.insert(0, "/opt/trn_rl_repo")
os.environ.setdefault("MYCRO_LOCAL_CACHE", "1")

import numpy as np
import ml_dtypes
import concourse.bass as bass  # noqa: F401  (Bass types via bacc)
import concourse.mybir as mybir
import concourse.tile as tile
from concourse import bacc, dve_ops
from concourse.dve_spec import Spec, Src0, C0, C1, C2, C3, One, sq, _spill_c3_to_src1
from concourse.bass_utils import run_bass_kernel_spmd
from contextlib import ExitStack

f32 = mybir.dt.float32
bf16 = mybir.dt.bfloat16
fp8 = mybir.dt.float8e4
AF = mybir.ActivationFunctionType
MUL = mybir.AluOpType.mult
DRMODE = mybir.MatmulPerfMode.DoubleRow

B, S, D = 4, 2048, 1024
H = 16
DH = 64
NCORES = 8
G_HEADS = 512  # head dims per core (8 heads)

EXP_B = 1.0 / 128.0  # inner poly scale: exp(x/8) = (e^(x/128))^16

# ---- custom DVE ops: two-op exp chain --------------------------------------
_t1 = Src0 * C0 + C1
_t2 = _t1 * Src0 + C2
_t3 = _t2 * Src0 + C3
_t4 = _t3 * Src0 + One


def _ref_expp(in0, in1, s0, s1, imm2):
    x = in0.astype(np.float32)
    return (((x * s0 + s1) * x + imm2) * x + in1) * x + 1.0


EXPP_ANT = dve_ops.DveOp(
    "EXPP_ANT",
    Spec(body=_spill_c3_to_src1(_t4), reference=_ref_expp),
    subdim=False,
    uops_sha={"v3": "728e43d6680666f6", "v4": "9a9d1d3477880b00"},
)


def _ref_pow16s(in0, in1, s0, s1, imm2):
    t = in0.astype(np.float32)
    t = t * t
    t = t * t
    t = t * t
    t = t * t
    return t * s0


POW16S_ANT = dve_ops.DveOp(
    "POW16S_ANT",
    Spec(body=sq(sq(sq(sq(Src0)))) * C0, reference=_ref_pow16s),
    subdim=False,
    uops_sha={"v3": "dc10736d1c0a5ecc", "v4": "4d740a20ba0e2e80"},
)

for _op in (EXPP_ANT, POW16S_ANT):
    if _op.name not in dve_ops._SUB_OPCODE_FOR_NAME:
        dve_ops.OPS.append(_op)
        dve_ops.CUSTOM_DVE_SPECS[_op.name] = _op.spec
        dve_ops._SUB_OPCODE_FOR_NAME[_op.name] = (
            max(dve_ops._SUB_OPCODE_FOR_NAME.values()) + 1
        )

DVE_KC = ()  # kc tiles whose exp runs on the vector engine (disabled: FIFO latency stalls)


def build():
    nc = bacc.Bacc(None, target_bir_lowering=False)

    QT = nc.dram_tensor("QT", [D, S], bf16, kind="ExternalInput")
    KT = nc.dram_tensor("KT", [D, S], bf16, kind="ExternalInput")
    VT = nc.dram_tensor("VT", [D, S], bf16, kind="ExternalInput")
    WqT = nc.dram_tensor("WqT", [D, G_HEADS], bf16, kind="ExternalInput")
    WkT = nc.dram_tensor("WkT", [D, G_HEADS], bf16, kind="ExternalInput")
    WvT = nc.dram_tensor("WvT", [D, G_HEADS], bf16, kind="ExternalInput")
    WoT = nc.dram_tensor("WoT", [G_HEADS, D], bf16, kind="ExternalInput")
    bqp = nc.dram_tensor("bqp", [128, 4], f32, kind="ExternalInput")
    bkp = nc.dram_tensor("bkp", [128, 4], f32, kind="ExternalInput")
    Y = nc.dram_tensor("Y", [S, D], bf16, kind="ExternalOutput")
    dbg = os.environ.get("KERNEL_DEBUG") == "1"
    if dbg:
        Dqq = nc.dram_tensor("Dqq", [128, 512], bf16, kind="ExternalOutput")
        DkT = nc.dram_tensor("DkT", [128, 512], bf16, kind="ExternalOutput")
        Dv8 = nc.dram_tensor("Dv8", [128, 640], bf16, kind="ExternalOutput")

        Drcs = nc.dram_tensor("Drcs", [1, 512], f32, kind="ExternalOutput")
        DAT = nc.dram_tensor("DAT", [128, 512], bf16, kind="ExternalOutput")

    with tile.TileContext(nc) as tc, ExitStack() as top:
        per = top.enter_context(tc.tile_pool(name="per", bufs=1))
        wq = top.enter_context(tc.tile_pool(name="wq", bufs=1))
        xqp = top.enter_context(tc.tile_pool(name="xq", bufs=2))
        cst = top.enter_context(tc.tile_pool(name="cst", bufs=1))
        ppool = top.enter_context(tc.tile_pool(name="pP", bufs=3))
        epool = top.enter_context(tc.tile_pool(name="eP", bufs=2))
        rcp = top.enter_context(tc.tile_pool(name="rcp", bufs=2))
        oev = top.enter_context(tc.tile_pool(name="oev", bufs=2))
        osc = top.enter_context(tc.tile_pool(name="osc", bufs=2))
        yev = top.enter_context(tc.tile_pool(name="yev", bufs=3))
        drp = top.enter_context(tc.tile_pool(name="drp", bufs=4, space="DRAM"))
        pps = top.enter_context(tc.tile_pool(name="pps", bufs=2, space="PSUM"))
        spool = top.enter_context(tc.tile_pool(name="sS", bufs=2, space="PSUM"))
        opool = top.enter_context(tc.tile_pool(name="sO", bufs=2, space="PSUM"))

        # persistent results of the projections
        kT_t = [per.tile([128, S], bf16, tag=f"kT{i}", name=f"kT{i}") for i in range(4)]
        v_st = [
            per.tile([128, 8 * 65], bf16, tag=f"v{i}", name=f"v{i}")
            for i in range(16)
        ]
        qq_t = [
            [per.tile([128, 512], bf16, tag=f"qq{i}_{j}", name=f"qq{i}_{j}") for j in range(4)]
            for i in range(4)
        ]
        AT_q = [
            [per.tile([128, 512], bf16, tag=f"AT{i}_{j}", name=f"AT{i}_{j}") for j in range(4)]
            for i in range(4)
        ]

        bq_sb = cst.tile([128, 4], f32, tag="bq")
        bk_sb = cst.tile([128, 4], f32, tag="bk")
        cexp = cst.tile([128, 1], f32, tag="cexp")
        nc.vector.memset(cexp[:], EXP_B)
        bneg1 = cst.tile([128, 1], f32, tag="bneg1")
        nc.vector.memset(bneg1[:], -5.0)
        Wk_sb = wq.tile([128, 8, G_HEADS], bf16, tag="Wk")
        Wq_sb = wq.tile([128, 8, G_HEADS], bf16, tag="Wq")
        Wv_sb = wq.tile([128, 8, G_HEADS], bf16, tag="Wv")
        Wo_sb = wq.tile([128, 4, D], bf16, tag="Wo")

        ksrc = KT.ap().rearrange("(d p) s -> p d s", p=128)
        qsrc = QT.ap().rearrange("(d p) s -> p d s", p=128)
        vsrc = VT.ap().rearrange("(d p) s -> p d s", p=128)

        # K path on the sync queue first: it gates the first matmul
        wksrc = WkT.ap().rearrange("(d p) c -> p d c", p=128)
        for wc in range(4):
            nc.sync.dma_start(
                Wk_sb[:, :, wc * 128 : (wc + 1) * 128],
                wksrc[:, :, wc * 128 : (wc + 1) * 128],
            )
        # weights + biases + first Q quarter on the gpsimd queue (parallel)
        nc.gpsimd.dma_start(bq_sb[:], bqp[:, :])
        nc.gpsimd.dma_start(bk_sb[:], bkp[:, :])
        nc.gpsimd.dma_start(Wq_sb[:], WqT.ap().rearrange("(d p) c -> p d c", p=128))
        qx0 = xqp.tile([128, 8, 512], bf16, tag="qx")
        nc.gpsimd.dma_start(qx0[:], qsrc[:, :, 0:512])
        nc.gpsimd.dma_start(Wv_sb[:], WvT.ap().rearrange("(d p) c -> p d c", p=128))
        nc.gpsimd.dma_start(Wo_sb[:], WoT.ap().rearrange("(g p) n -> p g n", p=128))

        # warm the exp table set early (one-time ~2.7us load)
        warm = cst.tile([128, 8], f32, tag="warm")
        nc.vector.memset(warm[:], 0.0)
        nc.scalar.activation(warm[:], warm[:], AF.Exp)

        # ---- K projection (kx transient; freed before vx allocates) ----
        with tc.tile_pool(name="kx", bufs=1) as kxp:
            kx = kxp.tile([128, 8, S], bf16, tag="kx")
            for sc in range(4):
                nc.sync.dma_start(
                    kx[:, :, sc * 512 : (sc + 1) * 512],
                    ksrc[:, :, sc * 512 : (sc + 1) * 512],
                )
            for sc in range(4):
                for hp in range(4):
                    ps = pps.tile([128, 512], f32, tag="ps")
                    for dc in range(8):
                        nc.tensor.matmul(
                            ps[:],
                            Wk_sb[:, dc, hp * 128 : (hp + 1) * 128],
                            kx[:, dc, sc * 512 : (sc + 1) * 512],
                            start=(dc == 0),
                            stop=(dc == 7),
                        )
                    nc.vector.tensor_scalar_add(
                        kT_t[hp][:, sc * 512 : (sc + 1) * 512], ps[:], bk_sb[:, hp : hp + 1]
                    )

            # Q quarter 0
            for hp in range(4):
                ps = pps.tile([128, 512], f32, tag="ps")
                for dc in range(8):
                    nc.tensor.matmul(
                        ps[:],
                        Wq_sb[:, dc, hp * 128 : (hp + 1) * 128],
                        qx0[:, dc, :],
                        start=(dc == 0),
                        stop=(dc == 7),
                    )
                nc.vector.tensor_scalar_add(
                    qq_t[hp][0][:], ps[:], bq_sb[:, hp : hp + 1]
                )

        # ---- V projection (vx transient) ----
        with tc.tile_pool(name="vx", bufs=1) as vxp:
            vx = vxp.tile([128, 8, S], bf16, tag="vx")
            for half in range(2):
                nc.sync.dma_start(
                    vx[:, :, half * 1024 : (half + 1) * 1024],
                    vsrc[:, :, half * 1024 : (half + 1) * 1024],
                )
            for st in range(16):
                ps = pps.tile([128, 512], f32, tag="ps")
                for dc in range(8):
                    nc.tensor.matmul(
                        ps[:],
                        vx[:, dc, st * 128 : (st + 1) * 128],
                        Wv_sb[:, dc, :],
                        start=(dc == 0),
                        stop=(dc == 7),
                    )
                vd = v_st[st][:].rearrange("p (h c) -> p h c", c=65)
                nc.vector.tensor_copy(
                    vd[:, :, 0:64], ps[:].rearrange("p (h c) -> p h c", c=64)
                )
                nc.vector.memset(vd[:, :, 64:65], 1.0)

        # ---- attention, quarter-outer, with projection work interleaved ----
        qx_t = {0: qx0}
        heldps = {}

        def emit_qproj_half(quarter, hpc, half):
            if half == 0:
                heldps[hpc] = pps.tile([128, 512], f32, tag="ps", name="psq")
            ps = heldps[hpc]
            qxt = qx_t[quarter]
            for dc in range(4 * half, 4 * half + 4):
                nc.tensor.matmul(
                    ps[:],
                    Wq_sb[:, dc, hpc * 128 : (hpc + 1) * 128],
                    qxt[:, dc, :],
                    start=(dc == 0),
                    stop=(dc == 7),
                )
            if half == 1:
                nc.vector.tensor_scalar_add(
                    qq_t[hpc][quarter][:], ps[:], bq_sb[:, hpc : hpc + 1]
                )

        def emit_outproj_chain(qp, chain):
            # chain in 0..7 -> (q4, nh)
            q4, nh = chain // 2, chain % 2
            qt_g = qp * 4 + q4
            ps = pps.tile([128, 512], f32, tag="ps")
            for hp in range(4):
                nc.tensor.matmul(
                    ps[:],
                    AT_q[hp][qp][:, q4 * 128 : (q4 + 1) * 128],
                    Wo_sb[:, hp, nh * 512 : (nh + 1) * 512],
                    start=(hp == 0),
                    stop=(hp == 3),
                )
            ye = yev.tile([128, 512], bf16, tag="ye")
            nc.vector.tensor_copy(ye[:], ps[:])
            nc.sync.dma_start(
                Y[qt_g * 128 : (qt_g + 1) * 128, nh * 512 : (nh + 1) * 512], ye[:]
            )

        for qp in range(4):
            # prefetch the next Q quarter's input slab
            if qp < 3:
                qx_t[qp + 1] = xqp.tile(
                    [128, 8, 512], bf16, tag="qx", name=f"qx{qp + 1}"
                )
                nc.gpsimd.dma_start(
                    qx_t[qp + 1][:], qsrc[:, :, (qp + 1) * 512 : (qp + 2) * 512]
                )
            for hp in range(4):
                kt = kT_t[hp]
                qtile = qq_t[hp][qp]
                O_t = [
                    opool.tile([128, 512], f32, tag="O", name="O0"),
                    opool.tile([128, 512], f32, tag="O", name="O1"),
                ]
                for kc in range(16):
                    S_big = spool.tile([128, 1024], f32, tag="S", name="S")
                    for hloc in range(2):
                        nc.tensor.matmul(
                            S_big[:, hloc * 512 : (hloc + 1) * 512],
                            kt[hloc * 64 : hloc * 64 + 64, kc * 128 : (kc + 1) * 128],
                            qtile[hloc * 64 : hloc * 64 + 64, :],
                            start=True,
                            stop=True,
                        )
                    P_big = ppool.tile([128, 1024], bf16, tag="P", name="P")
                    nc.scalar.activation(P_big[:], S_big[:], AF.Exp, scale=0.125)
                    for hloc in range(2):
                        lv = v_st[kc][:, (2 * hp + hloc) * 65 : (2 * hp + hloc) * 65 + 65]
                        nc.tensor.matmul(
                            O_t[hloc][0:65, :],
                            lv,
                            P_big[:, hloc * 512 : (hloc + 1) * 512],
                            start=(kc == 0),
                            stop=(kc == 15),
                        )
                    # interleave projection work to keep the scalar engine fed
                    if qp == 0:
                        if kc == 5:
                            emit_qproj_half(1, hp, 0)
                        elif kc == 13:
                            emit_qproj_half(1, hp, 1)
                    else:
                        if kc == 2:
                            emit_outproj_chain(qp - 1, 2 * hp)
                        elif kc == 10:
                            emit_outproj_chain(qp - 1, 2 * hp + 1)
                        if qp < 3:
                            if kc == 6:
                                emit_qproj_half(qp + 1, hp, 0)
                            elif kc == 14:
                                emit_qproj_half(qp + 1, hp, 1)
                # normalize: evict O (frees PSUM), round-trip the denominator
                # through DRAM as a 64-partition broadcast, reciprocal, multiply
                for hloc in range(2):
                    O = O_t[hloc]
                    ov = oev.tile([128, 512], f32, tag="ov")
                    nc.vector.tensor_copy(ov[0:65, :], O[0:65, :])
                    dr_t = drp.tile([1, 512], f32)
                    nc.sync.dma_start(dr_t[:, :], ov[64:65, :])
                    db_t = rcp.tile([128, 512], f32, tag="db")
                    nc.sync.dma_start(
                        db_t[0:64, :], dr_t[0:1, :].to_broadcast([64, 512])
                    )
                    if dbg and qp == 0 and hp == 0 and hloc == 0:
                        nc.sync.dma_start(Drcs.ap(), db_t[0:1, :])
                    rcb = rcp.tile([128, 512], f32, tag="rcb")
                    nc.vector.reciprocal_approx_fast(rcb[0:64, :], db_t[0:64, :])
                    if hloc == 0:
                        nc.vector.tensor_tensor(
                            AT_q[hp][qp][0:64, :], ov[0:64, :], rcb[0:64, :], MUL
                        )
                    else:
                        sct = osc.tile([128, 512], bf16, tag="osc")
                        nc.vector.tensor_tensor(
                            sct[0:64, :], ov[0:64, :], rcb[0:64, :], MUL
                        )
                        nc.sync.dma_start(AT_q[hp][qp][64:128, :], sct[0:64, :])
        if dbg:
            nc.sync.dma_start(Dqq.ap(), qq_t[0][0][:])
            nc.sync.dma_start(DkT.ap(), kT_t[0][:, 0:512])
            nc.sync.dma_start(
                Dv8.ap(), v_st[0][:, 0:640].rearrange("p n -> p n")
            )
            nc.sync.dma_start(DAT.ap(), AT_q[0][0][:])
        # last quarter's output projection
        for chain in range(8):
            emit_outproj_chain(3, chain)

    nc.compile()
    return nc


_NC = None


def _get_nc():
    global _NC
    if _NC is None:
        _NC = build()
    return _NC


def _prep_core(Q, K, V, Wq, bq, Wk, bk, Wv, Wo, b, g):
    c = np.ascontiguousarray
    bf = ml_dtypes.bfloat16
    hs = slice(g * G_HEADS, (g + 1) * G_HEADS)
    return {
        "QT": c(Q[b].T.astype(bf)),
        "KT": c(K[b].T.astype(bf)),
        "VT": c(V[b].T.astype(bf)),
        "WqT": c(Wq[hs, :].T.astype(bf)),
        "WkT": c(Wk[hs, :].T.astype(bf)),
        "WvT": c(Wv[hs, :].T.astype(bf)),
        "WoT": c(Wo[:, hs].T.astype(bf)),
        "bqp": c(bq[hs].reshape(4, 128).T),
        "bkp": c(bk[hs].reshape(4, 128).T),
    }


def kernel(Q, K, V, Wq, bq, Wk, bk, Wv, bv, Wo, bo, _want_trace=False):
    Q, K, V = (np.asarray(x, np.float32) for x in (Q, K, V))
    Wq, bq, Wk, bk, Wv, bv, Wo, bo = (
        np.asarray(x, np.float32) for x in (Wq, bq, Wk, bk, Wv, bv, Wo, bo)
    )
    nc = _get_nc()
    in_maps = [
        _prep_core(Q, K, V, Wq, bq, Wk, bk, Wv, Wo, b=c % 4, g=c // 4)
        for c in range(NCORES)
    ]
    res = run_bass_kernel_spmd(
        nc, in_maps, core_ids=list(range(NCORES)), trace=_want_trace
    )
    out = np.zeros((B, S, D), np.float32)
    for c in range(NCORES):
        out[c % 4] += res.results[c]["Y"].astype(np.float32)
    out += (bo + Wo.astype(np.float64) @ bv.astype(np.float64)).astype(np.float32)[
        None, None, :
    ]
    if _want_trace:
        kernel.last_exec_time_ns = res.exec_time_ns
        kernel.last_trace = res.instructions_and_trace
    return out


# revision 14
# speedup vs baseline: 1.2810x; 1.0263x over previous
"""Multi-head attention (B=4, S=2048, D=1024, H=16) on 8 Trainium2 cores.

Sharding: DP=4 over batch x TP=2 over heads (8 heads/core). Each core:
  - loads X^T inputs once as big-element DMAs (bf16), projects Q/K/V into
    per-head-pair transposed layouts (qT/kT: [dh, S] with dh on partitions;
    v: [S, dh])
  - flash-style attention without max-subtraction (scores ~ N(0,1)):
    S^T = kT.T-chunks @ qT (row-group-concurrent pairs of heads, K=64 each),
    P^T = exp(S^T/8) in bf16 on the scalar engine's exp LUT (fp8 P/v were
    tried and are categorically too noisy: every tensor here is a
    normalized Gaussian, so matmul outputs carry the full per-element
    relative quantization noise of their inputs — fp8's ~3.6% exceeds the
    accuracy budget while bf16's ~0.4% fits).
    O^T = [v | 1].T @ P^T (the ones column fuses the softmax denominator
    into row 64 of the PV output), normalize by evicting O to SBUF,
    round-tripping the denominator row through DRAM as a broadcast,
    reciprocal on 64 partitions (the 1-partition fast-reciprocal is
    broken on HW), then multiply
  - output projection partial Y_g = A_g @ Wo_g.T (bf16), Y emitted bf16
  - Q quarters 1-3 and the previous quarter's output projection are
    emitted interleaved into the exp-bound attention loop so the Tile
    scheduler cannot starve the scalar engine with projection bursts.
Host sums the two TP partials per batch and adds bo + Wo @ bv (the v-bias
commutes through the normalized softmax).
"""

import os
import sys

sys.path.insert(0, "/opt/trn_rl_repo")
os.environ.setdefault("MYCRO_LOCAL_CACHE", "1")

import numpy as np
import ml_dtypes
import concourse.bass as bass  # noqa: F401  (Bass types via bacc)
import concourse.mybir as mybir
import concourse.tile as tile
from concourse import bacc, dve_ops
from concourse.dve_spec import Spec, Src0, C0, C1, C2, C3, One, sq, _spill_c3_to_src1
from concourse.bass_utils import run_bass_kernel_spmd
from contextlib import ExitStack

f32 = mybir.dt.float32
bf16 = mybir.dt.bfloat16
fp8 = mybir.dt.float8e4
AF = mybir.ActivationFunctionType
MUL = mybir.AluOpType.mult
DRMODE = mybir.MatmulPerfMode.DoubleRow

B, S, D = 4, 2048, 1024
H = 16
DH = 64
NCORES = 8
G_HEADS = 512  # head dims per core (8 heads)

EXP_B = 1.0 / 128.0  # inner poly scale: exp(x/8) = (e^(x/128))^16

# ---- custom DVE ops: two-op exp chain --------------------------------------
_t1 = Src0 * C0 + C1
_t2 = _t1 * Src0 + C2
_t3 = _t2 * Src0 + C3
_t4 = _t3 * Src0 + One


def _ref_expp(in0, in1, s0, s1, imm2):
    x = in0.astype(np.float32)
    return (((x * s0 + s1) * x + imm2) * x + in1) * x + 1.0


EXPP_ANT = dve_ops.DveOp(
    "EXPP_ANT",
    Spec(body=_spill_c3_to_src1(_t4), reference=_ref_expp),
    subdim=False,
    uops_sha={"v3": "728e43d6680666f6", "v4": "9a9d1d3477880b00"},
)


def _ref_pow16s(in0, in1, s0, s1, imm2):
    t = in0.astype(np.float32)
    t = t * t
    t = t * t
    t = t * t
    t = t * t
    return t * s0


POW16S_ANT = dve_ops.DveOp(
    "POW16S_ANT",
    Spec(body=sq(sq(sq(sq(Src0)))) * C0, reference=_ref_pow16s),
    subdim=False,
    uops_sha={"v3": "dc10736d1c0a5ecc", "v4": "4d740a20ba0e2e80"},
)

for _op in (EXPP_ANT, POW16S_ANT):
    if _op.name not in dve_ops._SUB_OPCODE_FOR_NAME:
        dve_ops.OPS.append(_op)
        dve_ops.CUSTOM_DVE_SPECS[_op.name] = _op.spec
        dve_ops._SUB_OPCODE_FOR_NAME[_op.name] = (
            max(dve_ops._SUB_OPCODE_FOR_NAME.values()) + 1
        )

DVE_KC = ()  # kc tiles whose exp runs on the vector engine (disabled: FIFO latency stalls)


def build():
    nc = bacc.Bacc(None, target_bir_lowering=False)

    QT = nc.dram_tensor("QT", [D, S], bf16, kind="ExternalInput")
    KT = nc.dram_tensor("KT", [D, S], bf16, kind="ExternalInput")
    VT = nc.dram_tensor("VT", [D, S], bf16, kind="ExternalInput")
    WqT = nc.dram_tensor("WqT", [D, G_HEADS], bf16, kind="ExternalInput")
    WkT = nc.dram_tensor("WkT", [D, G_HEADS], bf16, kind="ExternalInput")
    WvT = nc.dram_tensor("WvT", [D, G_HEADS], bf16, kind="ExternalInput")
    WoT = nc.dram_tensor("WoT", [G_HEADS, D], bf16, kind="ExternalInput")
    bqp = nc.dram_tensor("bqp", [128, 4], f32, kind="ExternalInput")
    bkp = nc.dram_tensor("bkp", [128, 4], f32, kind="ExternalInput")
    Y = nc.dram_tensor("Y", [S, D], bf16, kind="ExternalOutput")
    dbg = os.environ.get("KERNEL_DEBUG") == "1"
    if dbg:
        Dqq = nc.dram_tensor("Dqq", [128, 512], bf16, kind="ExternalOutput")
        DkT = nc.dram_tensor("DkT", [128, 512], bf16, kind="ExternalOutput")
        Dv8 = nc.dram_tensor("Dv8", [128, 640], bf16, kind="ExternalOutput")

        Drcs = nc.dram_tensor("Drcs", [1, 512], f32, kind="ExternalOutput")
        DAT = nc.dram_tensor("DAT", [128, 512], bf16, kind="ExternalOutput")

    with tile.TileContext(nc) as tc, ExitStack() as top:
        per = top.enter_context(tc.tile_pool(name="per", bufs=1))
        wq = top.enter_context(tc.tile_pool(name="wq", bufs=1))
        xqp = top.enter_context(tc.tile_pool(name="xq", bufs=2))
        cst = top.enter_context(tc.tile_pool(name="cst", bufs=1))
        ppool = top.enter_context(tc.tile_pool(name="pP", bufs=3))
        epool = top.enter_context(tc.tile_pool(name="eP", bufs=2))
        rcp = top.enter_context(tc.tile_pool(name="rcp", bufs=2))
        oev = top.enter_context(tc.tile_pool(name="oev", bufs=2))
        osc = top.enter_context(tc.tile_pool(name="osc", bufs=2))
        yev = top.enter_context(tc.tile_pool(name="yev", bufs=3))
        drp = top.enter_context(tc.tile_pool(name="drp", bufs=4, space="DRAM"))
        pps = top.enter_context(tc.tile_pool(name="pps", bufs=2, space="PSUM"))
        spool = top.enter_context(tc.tile_pool(name="sS", bufs=2, space="PSUM"))
        opool = top.enter_context(tc.tile_pool(name="sO", bufs=2, space="PSUM"))

        # persistent results of the projections
        kT_t = [per.tile([128, S], bf16, tag=f"kT{i}", name=f"kT{i}") for i in range(4)]
        v_st = [
            per.tile([128, 8 * 65], bf16, tag=f"v{i}", name=f"v{i}")
            for i in range(16)
        ]
        qq_t = [
            [per.tile([128, 512], bf16, tag=f"qq{i}_{j}", name=f"qq{i}_{j}") for j in range(4)]
            for i in range(4)
        ]
        AT_q = [
            [per.tile([128, 512], bf16, tag=f"AT{i}_{j}", name=f"AT{i}_{j}") for j in range(4)]
            for i in range(4)
        ]

        bq_sb = cst.tile([128, 4], f32, tag="bq")
        bk_sb = cst.tile([128, 4], f32, tag="bk")
        cexp = cst.tile([128, 1], f32, tag="cexp")
        nc.vector.memset(cexp[:], EXP_B)
        bneg1 = cst.tile([128, 1], f32, tag="bneg1")
        nc.vector.memset(bneg1[:], -5.0)
        Wk_sb = wq.tile([128, 8, G_HEADS], bf16, tag="Wk")
        Wq_sb = wq.tile([128, 8, G_HEADS], bf16, tag="Wq")
        Wv_sb = wq.tile([128, 8, G_HEADS], bf16, tag="Wv")
        Wo_sb = wq.tile([128, 4, D], bf16, tag="Wo")

        ksrc = KT.ap().rearrange("(d p) s -> p d s", p=128)
        qsrc = QT.ap().rearrange("(d p) s -> p d s", p=128)
        vsrc = VT.ap().rearrange("(d p) s -> p d s", p=128)

        # K path on the sync queue first: it gates the first matmul
        nc.sync.dma_start(Wk_sb[:], WkT.ap().rearrange("(d p) c -> p d c", p=128))
        # weights + biases + first Q quarter on the gpsimd queue (parallel)
        nc.gpsimd.dma_start(bq_sb[:], bqp[:, :])
        nc.gpsimd.dma_start(bk_sb[:], bkp[:, :])
        nc.gpsimd.dma_start(Wq_sb[:], WqT.ap().rearrange("(d p) c -> p d c", p=128))
        qx0 = xqp.tile([128, 8, 512], bf16, tag="qx")
        nc.gpsimd.dma_start(qx0[:], qsrc[:, :, 0:512])
        nc.gpsimd.dma_start(Wv_sb[:], WvT.ap().rearrange("(d p) c -> p d c", p=128))
        nc.gpsimd.dma_start(Wo_sb[:], WoT.ap().rearrange("(g p) n -> p g n", p=128))

        # warm the exp table set early (one-time ~2.7us load)
        warm = cst.tile([128, 8], f32, tag="warm")
        nc.vector.memset(warm[:], 0.0)
        nc.scalar.activation(warm[:], warm[:], AF.Exp)

        # ---- K projection (kx transient; freed before vx allocates) ----
        with tc.tile_pool(name="kx", bufs=1) as kxp:
            kx = kxp.tile([128, 8, S], bf16, tag="kx")
            for sc in range(4):
                nc.sync.dma_start(
                    kx[:, :, sc * 512 : (sc + 1) * 512],
                    ksrc[:, :, sc * 512 : (sc + 1) * 512],
                )
            for sc in range(4):
                for hp in range(4):
                    ps = pps.tile([128, 512], f32, tag="ps")
                    for dc in range(8):
                        nc.tensor.matmul(
                            ps[:],
                            Wk_sb[:, dc, hp * 128 : (hp + 1) * 128],
                            kx[:, dc, sc * 512 : (sc + 1) * 512],
                            start=(dc == 0),
                            stop=(dc == 7),
                        )
                    nc.vector.tensor_scalar_add(
                        kT_t[hp][:, sc * 512 : (sc + 1) * 512], ps[:], bk_sb[:, hp : hp + 1]
                    )

            # Q quarter 0
            for hp in range(4):
                ps = pps.tile([128, 512], f32, tag="ps")
                for dc in range(8):
                    nc.tensor.matmul(
                        ps[:],
                        Wq_sb[:, dc, hp * 128 : (hp + 1) * 128],
                        qx0[:, dc, :],
                        start=(dc == 0),
                        stop=(dc == 7),
                    )
                nc.vector.tensor_scalar_add(
                    qq_t[hp][0][:], ps[:], bq_sb[:, hp : hp + 1]
                )

        # ---- V projection (vx transient) ----
        with tc.tile_pool(name="vx", bufs=1) as vxp:
            vx = vxp.tile([128, 8, S], bf16, tag="vx")
            for half in range(2):
                nc.sync.dma_start(
                    vx[:, :, half * 1024 : (half + 1) * 1024],
                    vsrc[:, :, half * 1024 : (half + 1) * 1024],
                )
            for st in range(16):
                ps = pps.tile([128, 512], f32, tag="ps")
                for dc in range(8):
                    nc.tensor.matmul(
                        ps[:],
                        vx[:, dc, st * 128 : (st + 1) * 128],
                        Wv_sb[:, dc, :],
                        start=(dc == 0),
                        stop=(dc == 7),
                    )
                vd = v_st[st][:].rearrange("p (h c) -> p h c", c=65)
                nc.vector.tensor_copy(
                    vd[:, :, 0:64], ps[:].rearrange("p (h c) -> p h c", c=64)
                )
                nc.vector.memset(vd[:, :, 64:65], 1.0)

        # ---- attention, quarter-outer, with projection work interleaved ----
        qx_t = {0: qx0}
        heldps = {}

        def emit_qproj_half(quarter, hpc, half):
            if half == 0:
                heldps[hpc] = pps.tile([128, 512], f32, tag="ps", name="psq")
            ps = heldps[hpc]
            qxt = qx_t[quarter]
            for dc in range(4 * half, 4 * half + 4):
                nc.tensor.matmul(
                    ps[:],
                    Wq_sb[:, dc, hpc * 128 : (hpc + 1) * 128],
                    qxt[:, dc, :],
                    start=(dc == 0),
                    stop=(dc == 7),
                )
            if half == 1:
                nc.vector.tensor_scalar_add(
                    qq_t[hpc][quarter][:], ps[:], bq_sb[:, hpc : hpc + 1]
                )

        def emit_outproj_chain(qp, chain, tail=False):
            # chain in 0..7 -> (q4, nh)
            q4, nh = chain // 2, chain % 2
            qt_g = qp * 4 + q4
            ps = pps.tile([128, 512], f32, tag="ps")
            for hp in range(4):
                nc.tensor.matmul(
                    ps[:],
                    AT_q[hp][qp][:, q4 * 128 : (q4 + 1) * 128],
                    Wo_sb[:, hp, nh * 512 : (nh + 1) * 512],
                    start=(hp == 0),
                    stop=(hp == 3),
                )
            ye = yev.tile([128, 512], bf16, tag="ye")
            if tail:
                nc.scalar.activation(ye[:], ps[:], AF.Copy)
            else:
                nc.vector.tensor_copy(ye[:], ps[:])
            dma_eng = nc.gpsimd if tail else nc.sync
            dma_eng.dma_start(
                Y[qt_g * 128 : (qt_g + 1) * 128, nh * 512 : (nh + 1) * 512], ye[:]
            )

        for qp in range(4):
            # prefetch the next Q quarter's input slab
            if qp < 3:
                qx_t[qp + 1] = xqp.tile(
                    [128, 8, 512], bf16, tag="qx", name=f"qx{qp + 1}"
                )
                nc.gpsimd.dma_start(
                    qx_t[qp + 1][:], qsrc[:, :, (qp + 1) * 512 : (qp + 2) * 512]
                )
            for hp in range(4):
                kt = kT_t[hp]
                qtile = qq_t[hp][qp]
                O_t = [
                    opool.tile([128, 512], f32, tag="O", name="O0"),
                    opool.tile([128, 512], f32, tag="O", name="O1"),
                ]
                for kc in range(16):
                    S_big = spool.tile([128, 1024], f32, tag="S", name="S")
                    for hloc in range(2):
                        nc.tensor.matmul(
                            S_big[:, hloc * 512 : (hloc + 1) * 512],
                            kt[hloc * 64 : hloc * 64 + 64, kc * 128 : (kc + 1) * 128],
                            qtile[hloc * 64 : hloc * 64 + 64, :],
                            start=True,
                            stop=True,
                        )
                    P_big = ppool.tile([128, 1024], bf16, tag="P", name="P")
                    nc.scalar.activation(P_big[:], S_big[:], AF.Exp, scale=0.125)
                    for hloc in range(2):
                        lv = v_st[kc][:, (2 * hp + hloc) * 65 : (2 * hp + hloc) * 65 + 65]
                        nc.tensor.matmul(
                            O_t[hloc][0:65, :],
                            lv,
                            P_big[:, hloc * 512 : (hloc + 1) * 512],
                            start=(kc == 0),
                            stop=(kc == 15),
                        )
                    # interleave projection work to keep the scalar engine fed
                    if qp == 0:
                        if kc == 5:
                            emit_qproj_half(1, hp, 0)
                        elif kc == 13:
                            emit_qproj_half(1, hp, 1)
                    else:
                        if kc == 2:
                            emit_outproj_chain(qp - 1, 2 * hp)
                        elif kc == 10:
                            emit_outproj_chain(qp - 1, 2 * hp + 1)
                        if qp < 3:
                            if kc == 6:
                                emit_qproj_half(qp + 1, hp, 0)
                            elif kc == 14:
                                emit_qproj_half(qp + 1, hp, 1)
                # normalize: evict O (frees PSUM), round-trip the denominator
                # through DRAM as a 64-partition broadcast, reciprocal, multiply
                for hloc in range(2):
                    O = O_t[hloc]
                    ov = oev.tile([128, 512], f32, tag="ov")
                    nc.vector.tensor_copy(ov[0:65, :], O[0:65, :])
                    dr_t = drp.tile([1, 512], f32)
                    nc.sync.dma_start(dr_t[:, :], ov[64:65, :])
                    db_t = rcp.tile([128, 512], f32, tag="db")
                    nc.sync.dma_start(
                        db_t[0:64, :], dr_t[0:1, :].to_broadcast([64, 512])
                    )
                    if dbg and qp == 0 and hp == 0 and hloc == 0:
                        nc.sync.dma_start(Drcs.ap(), db_t[0:1, :])
                    rcb = rcp.tile([128, 512], f32, tag="rcb")
                    nc.vector.reciprocal_approx_fast(rcb[0:64, :], db_t[0:64, :])
                    if hloc == 0:
                        nc.vector.tensor_tensor(
                            AT_q[hp][qp][0:64, :], ov[0:64, :], rcb[0:64, :], MUL
                        )
                    else:
                        sct = osc.tile([128, 512], bf16, tag="osc")
                        nc.vector.tensor_tensor(
                            sct[0:64, :], ov[0:64, :], rcb[0:64, :], MUL
                        )
                        nc.sync.dma_start(AT_q[hp][qp][64:128, :], sct[0:64, :])
        if dbg:
            nc.sync.dma_start(Dqq.ap(), qq_t[0][0][:])
            nc.sync.dma_start(DkT.ap(), kT_t[0][:, 0:512])
            nc.sync.dma_start(
                Dv8.ap(), v_st[0][:, 0:640].rearrange("p n -> p n")
            )
            nc.sync.dma_start(DAT.ap(), AT_q[0][0][:])
        # last quarter's output projection (tail: scalar-engine eviction,
        # gpsimd DMA queue — both idle by now)
        for chain in range(8):
            emit_outproj_chain(3, chain, tail=True)

    nc.compile()
    return nc


_NC = None


def _get_nc():
    global _NC
    if _NC is None:
        _NC = build()
    return _NC


def _prep_core(Q, K, V, Wq, bq, Wk, bk, Wv, Wo, b, g):
    c = np.ascontiguousarray
    bf = ml_dtypes.bfloat16
    hs = slice(g * G_HEADS, (g + 1) * G_HEADS)
    return {
        "QT": c(Q[b].T.astype(bf)),
        "KT": c(K[b].T.astype(bf)),
        "VT": c(V[b].T.astype(bf)),
        "WqT": c(Wq[hs, :].T.astype(bf)),
        "WkT": c(Wk[hs, :].T.astype(bf)),
        "WvT": c(Wv[hs, :].T.astype(bf)),
        "WoT": c(Wo[:, hs].T.astype(bf)),
        "bqp": c(bq[hs].reshape(4, 128).T),
        "bkp": c(bk[hs].reshape(4, 128).T),
    }


def kernel(Q, K, V, Wq, bq, Wk, bk, Wv, bv, Wo, bo, _want_trace=False):
    Q, K, V = (np.asarray(x, np.float32) for x in (Q, K, V))
    Wq, bq, Wk, bk, Wv, bv, Wo, bo = (
        np.asarray(x, np.float32) for x in (Wq, bq, Wk, bk, Wv, bv, Wo, bo)
    )
    nc = _get_nc()
    in_maps = [
        _prep_core(Q, K, V, Wq, bq, Wk, bk, Wv, Wo, b=c % 4, g=c // 4)
        for c in range(NCORES)
    ]
    res = run_bass_kernel_spmd(
        nc, in_maps, core_ids=list(range(NCORES)), trace=_want_trace
    )
    out = np.zeros((B, S, D), np.float32)
    for c in range(NCORES):
        out[c % 4] += res.results[c]["Y"].astype(np.float32)
    out += (bo + Wo.astype(np.float64) @ bv.astype(np.float64)).astype(np.float32)[
        None, None, :
    ]
    if _want_trace:
        kernel.last_exec_time_ns = res.exec_time_ns
        kernel.last_trace = res.instructions_and_trace
    return out
